# Initial kernel scaffold
#

# Trainium2 Bass kernel for nn_CustomAttention (cosine-sim multi-head attention).
#
# Sharding over 8 cores: core c handles batch b = c//2 and head group
# g = c%2 (8 of 16 heads).  Each core computes its heads' q/k/v projections
# (Megatron column-parallel), cosine-sim attention, and a partial output
# projection (row-parallel over the 512 feature columns it owns).  The host
# sums the two partial outputs per batch and adds out_b.
#
# Layouts on device (per core):
#   qT/kT: (dims=512, seq=1024) as 4 tiles of (128, 1024); dims on partitions
#          so the scores matmul contracts over head_dim on the partition axis.
#   scores are computed transposed, sT[k, q], so softmax's key-sum is a
#   PE matmul contraction; the ones-column appended to v gives the softmax
#   denominator for free (row 64 of the (65, q) p@v output).
#   l2-norm of k and the per-head logit scale fold into the exp() activation
#   scale (per-partition AP); q's norm is applied via a broadcast multiply.
#
# All heavy matmuls run in float32r (fp32 storage, ~1e-3 matmul accuracy at
# bf16 speed); the p@v pair runs in bf16 (probabilities are in [0, 1]).

import math

import numpy as np

import sys

sys.path.insert(0, "/opt/trn_rl_repo")

import concourse.bass as bass
import concourse.tile as tile
from concourse import bacc, mybir
from concourse.bass_utils import run_bass_kernel_spmd
from concourse.masks import make_identity

N = 1024  # sequence length
B = 4  # batch
C = 1024  # channels
H = 16  # total heads
HD = 64  # head dim
G = 512  # dims per core (8 heads)
NT = 4  # (128,1024) tiles of qT/kT per core
CC = 8  # contraction chunks of 128 over C
ST = 8  # seq tiles of 128
QC = 2  # seq chunks of 512
LOGIT_SCALE_MAX = math.log(1.0 / 0.01)

F32 = mybir.dt.float32
F32R = mybir.dt.float32r
BF16 = mybir.dt.bfloat16
AF = mybir.ActivationFunctionType

_CACHED_NC = None
_LAST_IN_MAPS = None


def build_nc():
    nc = bacc.Bacc("TRN2", target_bir_lowering=False)

    # fp32 data; declared float32r where consumed by fp32r matmuls (the PE
    # rounds internally; numpy side is plain float32 bits either way).
    qt_d = nc.declare_dram_parameter("qt", [C, N], F32R, isOutput=False)
    kt_d = nc.declare_dram_parameter("kt", [C, N], F32R, isOutput=False)
    vt_d = nc.declare_dram_parameter("vt", [C, N], F32R, isOutput=False)
    wq_d = nc.declare_dram_parameter("wq", [C, G], F32R, isOutput=False)
    wk_d = nc.declare_dram_parameter("wk", [C, G], F32R, isOutput=False)
    wv_d = nc.declare_dram_parameter("wv", [C, G], F32R, isOutput=False)
    wo_d = nc.declare_dram_parameter("wo", [G, C], F32R, isOutput=False)
    bq_d = nc.declare_dram_parameter("bq", [128, NT], F32, isOutput=False)
    bk_d = nc.declare_dram_parameter("bk", [128, NT], F32, isOutput=False)
    bv_d = nc.declare_dram_parameter("bv", [1, G], F32R, isOutput=False)
    sel8_d = nc.declare_dram_parameter("sel8", [NT, 128, 8], F32R, isOutput=False)
    ones1_d = nc.declare_dram_parameter("ones1", [1, 128], F32R, isOutput=False)
    sel2T_d = nc.declare_dram_parameter("sel2T", [2, 128], F32R, isOutput=False)
    lsinv2_d = nc.declare_dram_parameter("lsinv2", [8, 1], F32, isOutput=False)
    lsbias_d = nc.declare_dram_parameter("lsbias", [128, 8], F32, isOutput=False)
    out_d = nc.declare_dram_parameter("out", [N, C], F32, isOutput=True)
    rsq_dram = nc.dram_tensor("rsq_scratch", [8, N], F32)
    rq_dram = nc.dram_tensor("rq_scratch", [8, N], F32)

    with tile.TileContext(nc) as tc:
        with (
            tc.tile_pool(name="consts", bufs=1) as consts,
            tc.tile_pool(name="wo_p", bufs=1) as wo_p,
            tc.tile_pool(name="w_p", bufs=10) as w_p,
            tc.tile_pool(name="acts", bufs=9) as acts,
            tc.tile_pool(name="big", bufs=1) as big,
            tc.tile_pool(name="sq_p", bufs=2) as sq_p,
            tc.tile_pool(name="stats", bufs=1) as stats,
            tc.tile_pool(name="eT_p", bufs=2) as eT_p,
            tc.tile_pool(name="xu_p", bufs=2) as xu_p,
            tc.tile_pool(name="bc_p", bufs=2) as bc_p,
            tc.tile_pool(name="outs", bufs=3) as outs_p,
        ):
            qt_r = qt_d[:].rearrange("(cc p) n -> cc p n", p=128)
            kt_r = kt_d[:].rearrange("(cc p) n -> cc p n", p=128)
            vt_r = vt_d[:].rearrange("(cc p) n -> cc p n", p=128)
            wq_r = wq_d[:].rearrange("(cc p) g -> cc p g", p=128)
            wk_r = wk_d[:].rearrange("(cc p) g -> cc p g", p=128)
            wv_r = wv_d[:].rearrange("(cc p) g -> cc p g", p=128)

            # q-tensor chunks stream first: nothing sits ahead of them in the
            # DMA queues, so the first projection matmul starts ~2us in.
            pre_q = []
            for cc in range(CC):
                w_sb = w_p.tile([128, G], F32R, tag="w", name=f"wq{cc}")
                nc.sync.dma_start(out=w_sb[:], in_=wq_r[cc])
                a_sb = acts.tile([128, N], F32R, tag="act", name=f"aq{cc}")
                nc.sync.dma_start(out=a_sb[:], in_=qt_r[cc])
                pre_q.append((w_sb, a_sb))

            # ---- constants ----
            sel8 = consts.tile([128, NT, 8], F32R)
            nc.sync.dma_start(out=sel8[:], in_=sel8_d[:].rearrange("t p e -> p t e"))
            ones1 = consts.tile([1, 128], F32R)
            nc.sync.dma_start(out=ones1[:], in_=ones1_d[:])
            sel2T = consts.tile([2, 128], F32R)
            nc.sync.dma_start(out=sel2T[:], in_=sel2T_d[:])
            lsinv2 = consts.tile([8, 1], F32)
            nc.sync.dma_start(out=lsinv2[:], in_=lsinv2_d[:])
            lsbias = consts.tile([128, 8], F32)
            nc.sync.dma_start(out=lsbias[:], in_=lsbias_d[:])
            bq_sb = consts.tile([128, NT], F32)
            nc.sync.dma_start(out=bq_sb[:], in_=bq_d[:])
            bk_sb = consts.tile([128, NT], F32)
            nc.sync.dma_start(out=bk_sb[:], in_=bk_d[:])
            bv_sb = consts.tile([1, G], F32R)
            nc.sync.dma_start(out=bv_sb[:], in_=bv_d[:])
            ident8 = consts.tile([8, 8], F32)
            make_identity(nc, ident8[:])
            # ---- persistent big tiles ----
            qT = [big.tile([128, N], F32R, tag=f"qT{t}", name=f"qT{t}") for t in range(NT)]
            kT = [big.tile([128, N], F32R, tag=f"kT{t}", name=f"kT{t}") for t in range(NT)]
            v_sb = [big.tile([128, 8, HD + 1], F32R, tag=f"v{s}", name=f"v{s}") for s in range(ST)]
            xt = [big.tile([128, N], F32, tag=f"xt{t}", name=f"xt{t}") for t in range(NT)]
            rskT = stats.tile([128, ST, 8], F32)

            with (
                tc.tile_pool(name="pp", bufs=2, space="PSUM") as pp,
                tc.tile_pool(name="pv", bufs=2, space="PSUM") as pv,
                tc.tile_pool(name="pstat", bufs=2, space="PSUM") as pstat,
            ):
                # ======== q/k projections:  xT_t = (w_x^T chunk)^T @ actT ========
                for name, act_r, w_r, dst, b_sb in (
                    ("q", qt_r, wq_r, qT, bq_sb),
                    ("k", kt_r, wk_r, kT, bk_sb),
                ):
                    if name == "q":
                        w_ch = [p[0] for p in pre_q]
                        a_ch = [p[1] for p in pre_q]
                    else:
                        w_ch = []
                        a_ch = []
                        for cc in range(CC):
                            w_sb = w_p.tile([128, G], F32R, tag="w")
                            nc.sync.dma_start(out=w_sb[:], in_=w_r[cc])
                            a_sb = acts.tile([128, N], F32R, tag="act")
                            nc.sync.dma_start(out=a_sb[:], in_=act_r[cc])
                            w_ch.append(w_sb)
                            a_ch.append(a_sb)
                    for t in range(NT):
                        for qc in range(QC):
                            ps = pp.tile([128, 512], F32, tag="proj")
                            for cc in range(CC):
                                nc.tensor.matmul(
                                    ps[:],
                                    w_ch[cc][:, t * 128 : (t + 1) * 128],
                                    a_ch[cc][:, qc * 512 : (qc + 1) * 512],
                                    start=(cc == 0),
                                    stop=(cc == CC - 1),
                                )
                            # psum -> sbuf with per-dim bias add, f32r rounded
                            nc.vector.tensor_scalar_add(
                                out=dst[t][:, qc * 512 : (qc + 1) * 512],
                                in0=ps[:],
                                scalar1=b_sb[:, t : t + 1],
                            )
                        # squares for the ssq matmul
                        sq = sq_p.tile([128, N], F32R, tag="sq")
                        f32view = dst[t][:].bitcast(F32)
                        nc.vector.tensor_mul(out=sq[:], in0=f32view, in1=f32view)
                        # ssq rows accumulate into (8, N) psum via selector
                        if t == 0:
                            ps_ssq = pstat.tile([8, N], F32, tag="ssq", bufs=1)
                        for qc in range(QC):
                            nc.tensor.matmul(
                                ps_ssq[:, qc * 512 : (qc + 1) * 512],
                                sel8[:, t, :],
                                sq[:, qc * 512 : (qc + 1) * 512],
                                start=(t == 0),
                                stop=(t == NT - 1),
                            )
                    if name == "q":
                        # rsq_q = 1/sqrt(ssq)
                        rsq = stats.tile([8, N], F32, tag="rsq_q")
                        nc.scalar.activation(out=rsq[:], in_=ps_ssq[:], func=AF.Sqrt)
                        nc.vector.reciprocal(out=rsq[:], in_=rsq[:])
                        rsq_q = rsq
                    else:
                        # rsk = ls_h / sqrt(ssq)  (scale folds 1/ls^2)
                        rsk = stats.tile([8, N], F32, tag="rsk")
                        nc.scalar.activation(
                            out=rsk[:], in_=ps_ssq[:], func=AF.Sqrt,
                            bias=0.0, scale=lsinv2[:],
                        )
                        nc.vector.reciprocal(out=rsk[:], in_=rsk[:])

                # transpose rsk rows into per-key columns: (8, 128) -> (128, 8)
                for s in range(ST):
                    ps_t = pstat.tile([128, 8], F32, tag="rskT", bufs=2)
                    nc.tensor.transpose(
                        ps_t[:], rsk[:, s * 128 : (s + 1) * 128], ident8[:]
                    )
                    nc.vector.tensor_copy(out=rskT[:, s, :], in_=ps_t[:])

                # q-hat: multiply qT rows by broadcast 1/||q|| (per head)
                nc.sync.dma_start(out=rsq_dram[:], in_=rsq_q[:])
                for t in range(NT):
                    rqb = bc_p.tile([128, N], F32, tag="rqb")
                    for j in range(2):
                        h = 2 * t + j
                        nc.sync.dma_start(
                            out=rqb[j * 64 : (j + 1) * 64, :],
                            in_=rsq_dram[h : h + 1, :].to_broadcast((64, N)),
                        )
                    nc.vector.tensor_mul(
                        out=qT[t][:], in0=qT[t][:].bitcast(F32), in1=rqb[:]
                    )

                # ======== v projection (natural layout) + bias + ones col ========
                v_ch = []
                wv_ch = []
                for cc in range(CC):
                    wv_sb = w_p.tile([128, G], F32R, tag="w")
                    nc.sync.dma_start(out=wv_sb[:], in_=wv_r[cc])
                    va_sb = acts.tile([128, N], F32R, tag="act")
                    nc.sync.dma_start(out=va_sb[:], in_=vt_r[cc])
                    wv_ch.append(wv_sb)
                    v_ch.append(va_sb)
                for s in range(ST):
                    ps = pv.tile([128, G], F32, tag="vproj")
                    for cc in range(CC):
                        nc.tensor.matmul(
                            ps[:],
                            v_ch[cc][:, s * 128 : (s + 1) * 128],
                            wv_ch[cc][:],
                            start=(cc == 0),
                            stop=False,
                        )
                    # bias add via rank-1 matmul: ones(1,128)^T @ bv(1,512)
                    nc.tensor.matmul(ps[:], ones1[:], bv_sb[:], start=False, stop=True)
                    nc.vector.tensor_copy(
                        out=v_sb[s][:, :, 0:HD],
                        in_=ps[:].rearrange("p (h d) -> p h d", h=8),
                    )
                    nc.vector.memset(v_sb[s][:, :, HD].bitcast(F32), 1.0)

                # wo: after all projection inputs, well before the out-proj
                wo_sb = wo_p.tile([128, NT, C], F32R)
                nc.sync.dma_start(
                    out=wo_sb[:], in_=wo_d[:].rearrange("(t p) c -> p t c", p=128)
                )

            # ======== attention ========
            with (
                tc.tile_pool(name="psT", bufs=2, space="PSUM") as psT,
                tc.tile_pool(name="pxa", bufs=2, space="PSUM") as pxa,
            ):
                for t in range(NT):
                    xa = [
                        pxa.tile([65, 512], F32, tag=f"xa{j}{qc}", name=f"xa{j}{qc}", bufs=1)
                        for qc in range(QC)
                        for j in range(2)
                    ]
                    for s in range(ST):
                        for j in range(2):
                            h = 2 * t + j
                            # full-width scores for one key tile (two psum banks;
                            # each matmul writes one bank)
                            sT = psT.tile([128, N], F32, tag=f"sT{j}", bufs=1)
                            for qc in range(QC):
                                nc.tensor.matmul(
                                    sT[:, qc * 512 : (qc + 1) * 512],
                                    kT[t][j * 64 : (j + 1) * 64, s * 128 : (s + 1) * 128],
                                    qT[t][j * 64 : (j + 1) * 64, qc * 512 : (qc + 1) * 512],
                                    start=True,
                                    stop=True,
                                )
                            # one wide exp amortizes ACT's per-op overhead
                            eT = eT_p.tile([128, N], F32R, tag=f"eT{j}")
                            nc.scalar.activation(
                                out=eT[:], in_=sT[:], func=AF.Exp,
                                bias=lsbias[:, h : h + 1],
                                scale=rskT[:, s, h : h + 1],
                            )
                            for qc in range(QC):
                                nc.tensor.matmul(
                                    xa[2 * qc + j][:],
                                    v_sb[s][:, h, :],
                                    eT[:, qc * 512 : (qc + 1) * 512],
                                    start=(s == 0),
                                    stop=(s == ST - 1),
                                )
                    cst = stats.tile([2, N], F32, tag="cst", bufs=2, name=f"cst{t}")
                    for qc in range(QC):
                        for j in range(2):
                            h = 2 * t + j
                            xu = xu_p.tile([65, 512], F32, tag="xu")
                            nc.vector.tensor_copy(out=xu[:], in_=xa[2 * qc + j][:])
                            # softmax denominator row -> cst[j]
                            nc.sync.dma_start(
                                out=cst[j : j + 1, qc * 512 : (qc + 1) * 512],
                                in_=xu[64:65, :],
                            )
                            # numerator -> xt tile (partition shift for j=1)
                            nc.sync.dma_start(
                                out=xt[t][j * 64 : (j + 1) * 64, qc * 512 : (qc + 1) * 512],
                                in_=xu[0:64, :],
                            )

                    # per-tile normalization.  Interior tiles bounce 1/colsum
                    # through DRAM (latency hides under the next tile); the
                    # last tile uses a PE selector-matmul broadcast instead so
                    # the exposed tail stays short.
                    if t < NT - 1:
                        nc.vector.reciprocal(out=cst[:], in_=cst[:])
                        nc.sync.dma_start(
                            out=rq_dram[2 * t : 2 * t + 2, :], in_=cst[:]
                        )
                        rqc = bc_p.tile([128, N], F32, tag="rqb")
                        for j in range(2):
                            h = 2 * t + j
                            nc.sync.dma_start(
                                out=rqc[j * 64 : (j + 1) * 64, :],
                                in_=rq_dram[h : h + 1, :].to_broadcast((64, N)),
                            )
                        nc.vector.tensor_mul(
                            out=xt[t][:].bitcast(F32R), in0=xt[t][:], in1=rqc[:]
                        )
                    else:
                        with nc.allow_low_precision("f32r rounding of 1/colsum"):
                            nc.vector.reciprocal(
                                out=cst[:].bitcast(F32R), in_=cst[:]
                            )
                        for qc in range(QC):
                            rqc_ps = pxa.tile(
                                [128, 512], F32, tag=f"xa{0}{qc}",
                                name=f"rq{t}{qc}", bufs=1,
                            )
                            nc.tensor.matmul(
                                rqc_ps[:],
                                sel2T[:],
                                cst[:].bitcast(F32R)[:, qc * 512 : (qc + 1) * 512],
                                start=True,
                                stop=True,
                            )
                            nc.vector.tensor_mul(
                                out=xt[t][:, qc * 512 : (qc + 1) * 512].bitcast(F32R),
                                in0=xt[t][:, qc * 512 : (qc + 1) * 512],
                                in1=rqc_ps[:],
                            )

            # ======== output projection (partial over this core's 512 dims) ====
            with tc.tile_pool(name="po", bufs=4, space="PSUM") as po:
                for s in range(ST):
                    for coc in range(2):
                        ps = po.tile([128, 512], F32, tag="out")
                        for t in range(NT):
                            nc.tensor.matmul(
                                ps[:],
                                xt[t][:].bitcast(F32R)[:, s * 128 : (s + 1) * 128],
                                wo_sb[:, t, coc * 512 : (coc + 1) * 512],
                                start=(t == 0),
                                stop=(t == NT - 1),
                            )
                        o_sb = outs_p.tile([128, 512], F32, tag="osb")
                        nc.vector.tensor_copy(out=o_sb[:], in_=ps[:])
                        nc.sync.dma_start(
                            out=out_d[:][
                                s * 128 : (s + 1) * 128, coc * 512 : (coc + 1) * 512
                            ],
                            in_=o_sb[:],
                        )

    nc.compile()
    return nc


def kernel(
    query, key, value, in_proj_w, in_proj_b, logit_scale, out_w, out_b, **kw
):
    global _CACHED_NC
    query = np.asarray(query, dtype=np.float32)
    key = np.asarray(key, dtype=np.float32)
    value = np.asarray(value, dtype=np.float32)
    in_proj_w = np.asarray(in_proj_w, dtype=np.float32)
    in_proj_b = np.asarray(in_proj_b, dtype=np.float32)
    logit_scale = np.asarray(logit_scale, dtype=np.float32)
    out_w = np.asarray(out_w, dtype=np.float32)
    out_b = np.asarray(out_b, dtype=np.float32)

    ls = np.exp(np.minimum(logit_scale.reshape(H), LOGIT_SCALE_MAX))  # (16,)

    # selector constants: sel8[t, p, e] = 1 where e == head-slot of partition p
    sel8 = np.zeros((NT, 128, 8), dtype=np.float32)
    for t in range(NT):
        for p in range(128):
            sel8[t, p, 2 * t + p // 64] = 1.0

    sel2T_h = np.zeros((2, 128), dtype=np.float32)
    sel2T_h[0, 0:64] = 1.0
    sel2T_h[1, 64:128] = 1.0

    in_maps = []
    for c in range(8):
        b, g = c // 2, c % 2
        heads = slice(g * 8, (g + 1) * 8)
        dims = slice(g * G, (g + 1) * G)
        ls_c = ls[heads]  # (8,)
        qt = np.ascontiguousarray(query[:, b, :].T)
        kt = np.ascontiguousarray(key[:, b, :].T)
        vt = np.ascontiguousarray(value[:, b, :].T)
        wq = np.ascontiguousarray(in_proj_w[0 * C :, :][dims, :].T)
        wk = np.ascontiguousarray(in_proj_w[1 * C :, :][dims, :].T)
        wv = np.ascontiguousarray(in_proj_w[2 * C :, :][dims, :].T)
        wo = np.ascontiguousarray(out_w[:, dims].T)
        bq = np.ascontiguousarray(in_proj_b[0 * C :][dims].reshape(NT, 128).T)
        bk = np.ascontiguousarray(in_proj_b[1 * C :][dims].reshape(NT, 128).T)
        bv = in_proj_b[2 * C :][dims].reshape(1, G)
        lsinv2 = (1.0 / ls_c**2).reshape(8, 1)
        # per-(partition, head) exp bias: -ls_h, constant down partitions
        lsbias = np.repeat(-ls_c.reshape(1, 8), 128, axis=0)
        in_maps.append(
            {
                "qt": qt.copy(),
                "kt": kt.copy(),
                "vt": vt.copy(),
                "wq": wq.copy(),
                "wk": wk.copy(),
                "wv": wv.copy(),
                "wo": wo.copy(),
                "bq": bq.copy(),
                "bk": bk.copy(),
                "bv": np.ascontiguousarray(bv),
                "sel8": sel8,
                "ones1": np.ones((1, 128), dtype=np.float32),
                "sel2T": sel2T_h,
                "lsinv2": np.ascontiguousarray(lsinv2, dtype=np.float32),
                "lsbias": np.ascontiguousarray(lsbias, dtype=np.float32),
            }
        )

    global _LAST_IN_MAPS
    _LAST_IN_MAPS = in_maps
    if _CACHED_NC is None:
        _CACHED_NC = build_nc()
    res = run_bass_kernel_spmd(_CACHED_NC, in_maps, core_ids=list(range(8)))

    out = np.zeros((N, B, C), dtype=np.float32)
    for c in range(8):
        b = c // 2
        out[:, b, :] += res.results[c]["out"]
    out += out_b.reshape(1, 1, C)
    return out



# revision 23
# speedup vs baseline: 1.2290x; 1.2290x over previous

# Trainium2 Bass kernel for nn_CustomAttention (cosine-sim multi-head attention).
#
# Sharding over 8 cores: core c handles batch b = c//2 and head group
# g = c%2 (8 of 16 heads).  Each core computes its heads' q/k/v projections
# (Megatron column-parallel), cosine-sim attention, and a partial output
# projection (row-parallel over the 512 feature columns it owns).  The host
# sums the two partial outputs per batch and adds out_b.
#
# All streamed tensors are fp16 (halves DMA traffic at ~5e-4 element error;
# fp16 matmuls run 1 cycle/row at any free-dim size).  DMAs are merged into
# a few large strided transfers: the hardware descriptor generator (HWDGE)
# serializes ~625ns per DMA, so many small DMAs throttle the whole pipeline.
#
# Phase-merged schedule (the kernel is PE-bound overall, so the point is to
# keep the PE queue full from ~2us in):
#   - l2-norm stats are per head-PAIR (tile t), so attention head-pair 0
#     starts right after the q0/k0 projections (~18us), not after all
#     projections.
#   - the v projection and the remaining q/k projections are spread through
#     the early attention slots' spare PE cycles.
#   - attention runs one head-slot (t, j) at a time, exp() on ACT; the
#     slot's p@v (one PSUM bank-group per query tile -- PSUM accumulation is
#     bank-granular) is deferred two slots and spread in pairs through that
#     slot's exp stream, transposes later still, so the in-order PE queue
#     never starves.
#   - p@v is flipped: eT query-slices stationary, v (64 dims + ones column)
#     the 65-wide moving operand; output lands (query, dim) with the softmax
#     denominator per-partition -> reciprocal + scalar-mul normalize, then a
#     PE transpose restores (dim, query) for the output projection.
#   - the last slot drains per query tile with the output projection and
#     store interleaved.

import math

import numpy as np

import sys

sys.path.insert(0, "/opt/trn_rl_repo")

import concourse.bass as bass
import concourse.tile as tile
from concourse import bacc, mybir
from concourse.bass_utils import run_bass_kernel_spmd
from concourse.masks import make_identity

N = 1024  # sequence length
B = 4  # batch
C = 1024  # channels
H = 16  # total heads
HD = 64  # head dim
G = 512  # dims per core (8 heads)
NT = 4  # (128,1024) tiles of qT/kT per core
CC = 8  # contraction chunks of 128 over C
ST = 8  # seq tiles of 128
QC = 2  # seq chunks of 512
LOGIT_SCALE_MAX = math.log(1.0 / 0.01)

F32 = mybir.dt.float32
F32R = mybir.dt.float32r
FP16 = mybir.dt.float16
AF = mybir.ActivationFunctionType

# packed f32 constant columns: bq[0:4], bk[4:8], lsbias[8:16],
# per-pair lsinv2 [16:20] (rows 0-1)
CF32_W = 20
# packed fp16 constant columns: identT [0:128], sel2 [128:130]
CF16_W = 130

_CACHED_NC = None
_LAST_IN_MAPS = None
_RES = None
DEBUG = False


def build_nc():
    nc = bacc.Bacc("TRN2", target_bir_lowering=False)

    qt_d = nc.declare_dram_parameter("qt", [C, N], FP16, isOutput=False)
    kt_d = nc.declare_dram_parameter("kt", [C, N], FP16, isOutput=False)
    vt_d = nc.declare_dram_parameter("vt", [C, N], FP16, isOutput=False)
    wq_d = nc.declare_dram_parameter("wq", [C, G], FP16, isOutput=False)
    wk_d = nc.declare_dram_parameter("wk", [C, G], FP16, isOutput=False)
    wv_d = nc.declare_dram_parameter("wv", [C, G], FP16, isOutput=False)
    wo_d = nc.declare_dram_parameter("wo", [G, C], FP16, isOutput=False)
    cf32_d = nc.declare_dram_parameter("cf32", [128, CF32_W], F32, isOutput=False)
    cf16_d = nc.declare_dram_parameter("cf16", [128, CF16_W], FP16, isOutput=False)
    cfr_d = nc.declare_dram_parameter("cfr", [1, 640], F32R, isOutput=False)
    out_d = nc.declare_dram_parameter("out", [N, C], FP16, isOutput=True)
    rsq_dram = nc.dram_tensor("rsq_scratch", [8, N], FP16)
    if DEBUG:
        dbg_q0_d = nc.declare_dram_parameter("dbg_q0", [128, N], FP16, isOutput=True)
        dbg_e_d = nc.declare_dram_parameter("dbg_e000", [128, N], FP16, isOutput=True)
        dbg_xt_d = nc.declare_dram_parameter("dbg_xt0", [128, N], FP16, isOutput=True)

    with tile.TileContext(nc) as tc:
        with (
            tc.tile_pool(name="consts", bufs=1) as consts,
            tc.tile_pool(name="wo_p", bufs=1) as wo_p,
            tc.tile_pool(name="w_p", bufs=1) as w_p,
            tc.tile_pool(name="acts", bufs=1) as acts,
            tc.tile_pool(name="big", bufs=1) as big,
            tc.tile_pool(name="sq_p", bufs=2) as sq_p,
            tc.tile_pool(name="stats", bufs=1) as stats,
            tc.tile_pool(name="st32_p", bufs=2) as st32_p,
            tc.tile_pool(name="stf_p", bufs=2) as stf_p,
            tc.tile_pool(name="eT_p", bufs=24) as eT_p,
            tc.tile_pool(name="xh_p", bufs=8) as xh_p,
            tc.tile_pool(name="rc_p", bufs=8) as rc_p,
            tc.tile_pool(name="bc_p", bufs=4) as bc_p,
            tc.tile_pool(name="outs", bufs=3) as outs_p,
            tc.tile_pool(name="pw", bufs=2, space="PSUM") as pw,
        ):
            # (partition, chunk, col) views of the streamed tensors
            qt_r = qt_d[:].rearrange("(cc p) n -> p cc n", p=128)
            kt_r = kt_d[:].rearrange("(cc p) n -> p cc n", p=128)
            vt_r = vt_d[:].rearrange("(cc p) n -> p cc n", p=128)
            wq_r = wq_d[:].rearrange("(cc p) g -> p cc g", p=128)
            wk_r = wk_d[:].rearrange("(cc p) g -> p cc g", p=128)
            wv_r = wv_d[:].rearrange("(cc p) g -> p cc g", p=128)

            # few large DMAs (HWDGE pays ~625ns per DMA, serialized); the
            # rsq0/rqb0 bounce DMAs are emitted mid-stream so head-pair 0's
            # q-norm lands before attention starts
            wq_sb = w_p.tile([128, CC, G], FP16, tag="wq", name="wq_sb")
            nc.sync.dma_start(out=wq_sb[:], in_=wq_r)
            aq_sb = acts.tile([128, CC, N], FP16, tag="aq", name="aq_sb")
            nc.sync.dma_start(out=aq_sb[:], in_=qt_r)
            cf32 = consts.tile([128, CF32_W], F32)
            nc.sync.dma_start(out=cf32[:], in_=cf32_d[:])
            cfr = consts.tile([1, 640], F32R)
            nc.sync.dma_start(out=cfr[:], in_=cfr_d[:])
            wk_sb = w_p.tile([128, CC, G], FP16, tag="wk", name="wk_sb")
            nc.sync.dma_start(out=wk_sb[:], in_=wk_r)
            ak_sb = acts.tile([128, CC, N], FP16, tag="ak", name="ak_sb")
            nc.sync.dma_start(out=ak_sb[:], in_=kt_r)
            cf16 = consts.tile([128, CF16_W], FP16)
            nc.sync.dma_start(out=cf16[:], in_=cf16_d[:])

            ident8 = consts.tile([8, 8], F32)
            make_identity(nc, ident8[:])

            def bq_col(t):
                return cf32[:, t : t + 1]

            def bk_col(t):
                return cf32[:, 4 + t : 5 + t]

            def lsbias_col(h):
                return cf32[:, 8 + h : 9 + h]

            def lsinv2_t(t):
                return cf32[0:2, 16 + t : 17 + t]

            ones1 = cfr[0:1, 0:128]
            bv_row = cfr[0:1, 128:640]
            identT = cf16[:, 0:128]
            sel2 = cf16[:, 128:130]
            ident2 = ident8[0:2, 0:2]

            # ---- persistent big tiles ----
            qT = [big.tile([128, N], FP16, tag=f"qT{t}", name=f"qT{t}") for t in range(NT)]
            kT = [big.tile([128, N], FP16, tag=f"kT{t}", name=f"kT{t}") for t in range(NT)]
            v_sb = [big.tile([128, 8, HD + 1], FP16, tag=f"v{s}", name=f"v{s}") for s in range(ST)]
            xt = [big.tile([128, N], FP16, tag=f"xt{t}", name=f"xt{t}") for t in range(NT)]
            rskT = stats.tile([128, ST, 8], F32)

            # Shared PSUM: "sT" (2 bufs x 2 banks), "work" (2 x 1 bank:
            # p@v accumulators / transposes / rsk transposes), "proj"
            # (2 x 1 bank: projection groups, v groups, ssq rows)
            with (
                tc.tile_pool(name="psT", bufs=2, space="PSUM") as psT,
                tc.tile_pool(name="pxa", bufs=2, space="PSUM") as pxa,
            ):

                def proj_grp(w_sb, a_sb, t, qc, dst, b_col, nm):
                    ps = pw.tile([128, 512], F32, tag="proj", name=f"pp{nm}{t}{qc}")
                    for cc in range(CC):
                        nc.tensor.matmul(
                            ps[:],
                            w_sb[:, cc, t * 128 : (t + 1) * 128],
                            a_sb[:, cc, qc * 512 : (qc + 1) * 512],
                            start=(cc == 0),
                            stop=(cc == CC - 1),
                        )
                    nc.vector.tensor_scalar_add(
                        out=dst[t][:, qc * 512 : (qc + 1) * 512],
                        in0=ps[:],
                        scalar1=b_col,
                    )

                def ssq_pair(dst_t, t, nm):
                    # per-pair sum of squares -> (2, N) f32 in two qc halves
                    sq = sq_p.tile([128, N], FP16, tag="sq", name=f"sq{nm}{t}")
                    nc.vector.tensor_mul(out=sq[:], in0=dst_t[:], in1=dst_t[:])
                    s32 = st32_p.tile([2, N], F32, tag="s32", name=f"s32{nm}{t}")
                    for qc in range(QC):
                        ps = pw.tile(
                            [2, 512], F32, tag="proj", name=f"ssq{nm}{t}{qc}"
                        )
                        nc.tensor.matmul(
                            ps[:],
                            sel2,
                            sq[:, qc * 512 : (qc + 1) * 512],
                            start=True,
                            stop=True,
                        )
                        yield ps, s32, qc
                    # caller finishes sqrt per half into s32

                def stats_q(t):
                    for ps, s32, qc in ssq_pair(qT[t], t, "q"):
                        nc.scalar.activation(
                            out=s32[:, qc * 512 : (qc + 1) * 512], in_=ps[:],
                            func=AF.Sqrt,
                        )
                    nc.vector.reciprocal(out=s32[:], in_=s32[:])
                    sf = stf_p.tile([2, N], FP16, tag="sf", name=f"rsqf{t}")
                    nc.vector.tensor_copy(out=sf[:], in_=s32[:])
                    nc.sync.dma_start(
                        out=rsq_dram[2 * t : 2 * t + 2, :], in_=sf[:]
                    )
                    rqb = bc_p.tile(
                        [128, N], FP16, tag="rqb", name=f"rqb{t}", bufs=4
                    )
                    for j in range(2):
                        nc.sync.dma_start(
                            out=rqb[j * 64 : (j + 1) * 64, :],
                            in_=rsq_dram[2 * t + j : 2 * t + j + 1, :]
                            .to_broadcast((64, N)),
                        )
                    # q-hat: all-SBUF fp16 -> DVE 4x mode
                    nc.vector.tensor_mul(out=qT[t][:], in0=qT[t][:], in1=rqb[:])

                def stats_k(t):
                    for ps, s32, qc in ssq_pair(kT[t], t, "k"):
                        nc.scalar.activation(
                            out=s32[:, qc * 512 : (qc + 1) * 512], in_=ps[:],
                            func=AF.Sqrt, bias=0.0, scale=lsinv2_t(t),
                        )
                    nc.vector.reciprocal(out=s32[:], in_=s32[:])
                    # transpose rsk rows into per-key columns of rskT
                    for st in range(ST):
                        ps_t = pxa.tile(
                            [128, 2], F32, tag="work", bufs=2, name=f"rskT{t}{st}"
                        )
                        nc.tensor.transpose(
                            ps_t[:], s32[:, st * 128 : (st + 1) * 128], ident2
                        )
                        nc.vector.tensor_copy(
                            out=rskT[:, st, 2 * t : 2 * t + 2], in_=ps_t[:]
                        )

                def v_group(s):
                    ps = pw.tile([128, G], F32, tag="proj", name=f"pv{s}")
                    for cc in range(CC):
                        nc.tensor.matmul(
                            ps[:],
                            av_sb[:, cc, s * 128 : (s + 1) * 128],
                            wv_sb[:, cc, :],
                            start=(cc == 0),
                            stop=False,
                        )
                    # bias add via rank-1 matmul: ones(1,128)^T @ bv(1,512)
                    nc.tensor.matmul(ps[:], ones1, bv_row, start=False, stop=True)
                    nc.vector.tensor_copy(
                        out=v_sb[s][:, :, 0:HD],
                        in_=ps[:].rearrange("p (h d) -> p h d", h=8),
                    )
                    nc.gpsimd.memset(v_sb[s][:, :, HD], 1.0)

                # ---- pre-loop: pair-0 projections + stats ----
                for qc in range(QC):
                    proj_grp(wq_sb, aq_sb, 0, qc, qT, bq_col(0), "q")
                stats_q(0)
                for qc in range(QC):
                    proj_grp(wk_sb, ak_sb, 0, qc, kT, bk_col(0), "k")
                stats_k(0)

                # late input streams (behind the pair-0 bounce DMAs in SP
                # order so rqb0 isn't stuck behind them)
                wv_sb = w_p.tile([128, CC, G], FP16, tag="wv", name="wv_sb")
                nc.sync.dma_start(out=wv_sb[:], in_=wv_r)
                av_sb = acts.tile([128, CC, N], FP16, tag="av", name="av_sb")
                nc.sync.dma_start(out=av_sb[:], in_=vt_r)
                wo_sb = wo_p.tile([128, NT, C], FP16)
                nc.sync.dma_start(
                    out=wo_sb[:], in_=wo_d[:].rearrange("(t p) c -> p t c", p=128)
                )
                if DEBUG:
                    nc.sync.dma_start(out=dbg_q0_d[:], in_=qT[0][:])

                # deferred projection / v work, emitted one item per exp step
                # through the early attention slots
                extra = []
                for s in range(ST):
                    extra.append(lambda s=s: v_group(s))
                for tt in range(1, NT):
                    extra.append(
                        lambda tt=tt: proj_grp(wq_sb, aq_sb, tt, 0, qT, bq_col(tt), "q")
                    )

                    def qrest(tt=tt):
                        proj_grp(wq_sb, aq_sb, tt, 1, qT, bq_col(tt), "q")
                        stats_q(tt)

                    extra.append(qrest)
                    extra.append(
                        lambda tt=tt: proj_grp(wk_sb, ak_sb, tt, 0, kT, bk_col(tt), "k")
                    )

                    def krest(tt=tt):
                        proj_grp(wk_sb, ak_sb, tt, 1, kT, bk_col(tt), "k")
                        stats_k(tt)

                    extra.append(krest)
                extra = extra[::-1]  # pop from the end

                def pv_norm(t, j, eTs, qt):
                    h = 2 * t + j
                    acc = pxa.tile(
                        [128, HD + 1], F32, tag="work",
                        name=f"acc{t}{j}{qt}", bufs=2,
                    )
                    for s in range(ST):
                        nc.tensor.matmul(
                            acc[:],
                            eTs[s][:, qt * 128 : (qt + 1) * 128],
                            v_sb[s][:, h, :],
                            start=(s == 0),
                            stop=(s == ST - 1),
                        )
                    rc = rc_p.tile(
                        [128, 1], F32, tag="rc", name=f"rc{t}{j}{qt}", bufs=8
                    )
                    nc.vector.reciprocal(out=rc[:], in_=acc[:, HD : HD + 1])
                    xh = xh_p.tile(
                        [128, HD], FP16, tag="xh", name=f"xh{t}{j}{qt}", bufs=8
                    )
                    nc.vector.tensor_scalar_mul(
                        out=xh[:], in0=acc[:, 0:HD], scalar1=rc[:]
                    )
                    return xh

                def tp_block(t, j, xhs):
                    tp = pxa.tile(
                        [64, ST, 128], FP16, tag="work", name=f"tp{t}{j}", bufs=2
                    )
                    for qt in range(ST):
                        nc.tensor.transpose(tp[:, qt, :], xhs[qt][:], identT)
                    nc.vector.tensor_copy(
                        out=xt[t][j * 64 : (j + 1) * 64, :],
                        in_=tp[:].rearrange("p q n -> p (q n)"),
                    )
                    if DEBUG and t == 0 and j == 1:
                        nc.sync.dma_start(out=dbg_xt_d[:], in_=xt[0][:])

                def outproj(s, o_sb):
                    for coc in range(2):
                        ps = psT.tile(
                            [128, 512], F32, tag="sT", name=f"po{s}{coc}", bufs=2
                        )
                        for t in range(NT):
                            nc.tensor.matmul(
                                ps[:],
                                xt[t][:, s * 128 : (s + 1) * 128],
                                wo_sb[:, t, coc * 512 : (coc + 1) * 512],
                                start=(t == 0),
                                stop=(t == NT - 1),
                            )
                        if coc == 0:
                            nc.scalar.activation(
                                out=o_sb[:, 0:512], in_=ps[:], func=AF.Copy
                            )
                        else:
                            nc.vector.tensor_copy(out=o_sb[:, 512:1024], in_=ps[:])
                    nc.sync.dma_start(
                        out=out_d[:][s * 128 : (s + 1) * 128, :], in_=o_sb[:]
                    )

                # ---- attention slots ----
                slots = [(t, j) for t in range(NT) for j in range(2)]
                eT_hist = []
                xhs_hist = {}
                for idx, (t, j) in enumerate(slots):
                    h = 2 * t + j
                    eTs = []
                    for s in range(ST):
                        sT = psT.tile(
                            [128, N], F32, tag="sT", name=f"sT{t}{s}{j}", bufs=2
                        )
                        for qc in range(QC):
                            nc.tensor.matmul(
                                sT[:, qc * 512 : (qc + 1) * 512],
                                kT[t][j * 64 : (j + 1) * 64, s * 128 : (s + 1) * 128],
                                qT[t][j * 64 : (j + 1) * 64, qc * 512 : (qc + 1) * 512],
                                start=True,
                                stop=True,
                            )
                        eT = eT_p.tile(
                            [128, N], FP16, tag="eT", name=f"eT{t}{j}{s}", bufs=24
                        )
                        nc.scalar.activation(
                            out=eT[:], in_=sT[:], func=AF.Exp,
                            bias=lsbias_col(h),
                            scale=rskT[:, s, h : h + 1],
                        )
                        if DEBUG and idx == 0 and s == 0:
                            nc.sync.dma_start(out=dbg_e_d[:], in_=eT[:])
                        eTs.append(eT)
                        # deferred projection / v work
                        if extra and (idx == 0 and s >= 2 or idx in (1, 2)):
                            extra.pop()()
                        # p@v of slot idx-2, in pairs, transposes at s==6
                        if idx >= 2:
                            pt, pj = slots[idx - 2]
                            peTs = eT_hist[idx - 2]
                            if 1 <= s <= 4:
                                xhs = xhs_hist.setdefault(idx - 2, [])
                                xhs.append(pv_norm(pt, pj, peTs, 2 * (s - 1)))
                                xhs.append(pv_norm(pt, pj, peTs, 2 * s - 1))
                            if s == 6:
                                tp_block(pt, pj, xhs_hist[idx - 2])
                    eT_hist.append(eTs)

                # ---- drain: slots 6 and 7 ----
                pt, pj = slots[6]
                xhs6 = [pv_norm(pt, pj, eT_hist[6], qt) for qt in range(ST)]
                tp_block(pt, pj, xhs6)

                t, j = slots[7]
                eTs = eT_hist[7]
                tpd = pxa.tile(
                    [64, ST, 128], FP16, tag="work", name="tp_drain", bufs=2
                )

                def commit(qt, xh):
                    nc.tensor.transpose(tpd[:, qt, :], xh[:], identT)
                    nc.vector.tensor_copy(
                        out=xt[t][j * 64 : (j + 1) * 64, qt * 128 : (qt + 1) * 128],
                        in_=tpd[:, qt, :],
                    )
                    o_sb = outs_p.tile([128, N], FP16, tag="osb", name=f"osb{qt}")
                    outproj(qt, o_sb)

                xh_prev = pv_norm(t, j, eTs, 0)
                for qt in range(1, ST):
                    xh = pv_norm(t, j, eTs, qt)
                    commit(qt - 1, xh_prev)
                    xh_prev = xh
                commit(ST - 1, xh_prev)

    nc.compile()
    return nc


def kernel(
    query, key, value, in_proj_w, in_proj_b, logit_scale, out_w, out_b, **kw
):
    global _CACHED_NC
    query = np.asarray(query, dtype=np.float32)
    key = np.asarray(key, dtype=np.float32)
    value = np.asarray(value, dtype=np.float32)
    in_proj_w = np.asarray(in_proj_w, dtype=np.float32)
    in_proj_b = np.asarray(in_proj_b, dtype=np.float32)
    logit_scale = np.asarray(logit_scale, dtype=np.float32)
    out_w = np.asarray(out_w, dtype=np.float32)
    out_b = np.asarray(out_b, dtype=np.float32)

    f16 = np.float16
    ls = np.exp(np.minimum(logit_scale.reshape(H), LOGIT_SCALE_MAX))  # (16,)

    cf16 = np.zeros((128, CF16_W), dtype=f16)
    cf16[:, 0:128] = np.eye(128, dtype=f16)
    for p in range(128):
        cf16[p, 128 + p // 64] = 1.0

    in_maps = []
    for c in range(8):
        b, g = c // 2, c % 2
        heads = slice(g * 8, (g + 1) * 8)
        dims = slice(g * G, (g + 1) * G)
        ls_c = ls[heads]  # (8,)
        qt = np.ascontiguousarray(query[:, b, :].T.astype(f16))
        kt = np.ascontiguousarray(key[:, b, :].T.astype(f16))
        vt = np.ascontiguousarray(value[:, b, :].T.astype(f16))
        wq = np.ascontiguousarray(in_proj_w[0 * C :, :][dims, :].T.astype(f16))
        wk = np.ascontiguousarray(in_proj_w[1 * C :, :][dims, :].T.astype(f16))
        wv = np.ascontiguousarray(in_proj_w[2 * C :, :][dims, :].T.astype(f16))
        wo = np.ascontiguousarray(out_w[:, dims].T.astype(f16))

        cf32 = np.zeros((128, CF32_W), dtype=np.float32)
        cf32[:, 0:4] = in_proj_b[0 * C :][dims].reshape(NT, 128).T
        cf32[:, 4:8] = in_proj_b[1 * C :][dims].reshape(NT, 128).T
        cf32[:, 8:16] = np.repeat(-ls_c.reshape(1, 8), 128, axis=0)
        for t in range(NT):
            cf32[0:2, 16 + t] = 1.0 / ls_c[2 * t : 2 * t + 2] ** 2

        cfr = np.zeros((1, 640), dtype=np.float32)
        cfr[0, 0:128] = 1.0
        cfr[0, 128:640] = in_proj_b[2 * C :][dims]

        in_maps.append(
            {
                "qt": qt,
                "kt": kt,
                "vt": vt,
                "wq": wq,
                "wk": wk,
                "wv": wv,
                "wo": wo,
                "cf32": cf32,
                "cf16": cf16,
                "cfr": cfr,
            }
        )

    global _LAST_IN_MAPS, _RES
    _LAST_IN_MAPS = in_maps
    if _CACHED_NC is None:
        _CACHED_NC = build_nc()
    res = run_bass_kernel_spmd(_CACHED_NC, in_maps, core_ids=list(range(8)))
    _RES = res.results

    out = np.zeros((N, B, C), dtype=np.float32)
    for c in range(8):
        b = c // 2
        out[:, b, :] += res.results[c]["out"].astype(np.float32)
    out += out_b.reshape(1, 1, C)
    return out
